# revision 13
# baseline (speedup 1.0000x reference)
"""Trainium2 Bass kernel for nn_Attention_23424751632639.

Computation (per (b,h)):  out = tril_strict(rope(Q) @ rope(Q).T / sqrt(N)) @ V
Chunked linear attention (exact reordering of the sums):
  out_c = QR_c @ M_c  +  strict_mask(QR_c @ QR_c^T) @ V_c
  M_{c+1} = M_c + QR_c^T @ V_c          (M is the [64,64] running state)
with QR = rope(Q) * N**-0.25 (scale folded into the cos/sin tables).

v2 design vs baseline:
  * Host marshals Q, swap(Q), V to bf16 in chunk-major [128, ch*2*64]
    layout with 2 heads packed per tensor -> contiguous 4-8KB DMA lines
    (the fp32 rearrange DMAs were 256B/partition packets at ~110GB/s).
  * RoPE on device in bf16: DVE 2x-mode muls/adds + GpSimd for the
    second multiply; swap(Q) arrives pre-permuted so no strided copies.
  * QR^T strips produced by DMA-xbar transpose (dma_start_transpose of
    [128,128] 2-head feature blocks), not PE transposes + ACT evac.
  * intra/inter matmuls of the two packed heads use disjoint PE row
    groups (K=64 at base partitions 0/64) -> concurrent tiles.
  * state matmuls col-tiled into one [128,64] PSUM tile (head A on
    partitions 0:64, head B on 64:128) -> one snapshot copy per chunk
    serves both heads' inter matmuls.
  * mask applied as ACT PSUM->SBUF bf16 copy + GpSimd bf16 multiply.
  * outputs accumulate 4 chunks per PSUM bank; single DVE bf16 evac and
    one 512-col DMA per 4 chunks; output returned to HBM in bf16
    (tolerance is 2e-2; bf16 out adds ~4e-3).

Sharding: B*H = 32 (b,h) pairs -> 4 per core across 8 cores; no collectives.
"""

import math
import sys

import numpy as np

if "/opt/trn_rl_repo" not in sys.path:
    sys.path.insert(0, "/opt/trn_rl_repo")

import ml_dtypes

BF16 = ml_dtypes.bfloat16

B, H, T, N = 2, 16, 4096, 64
THETA = 2.0 ** 16
NCORES = 8
HPC = (B * H) // NCORES      # heads per core (4)
NPAIR = HPC // 2             # head pairs per core (2)
CH = T // 128                # 128-row chunks per head (32)
XBAR = True                  # transpose via DMA xbar (False -> PE transpose)


def _host_tables(t_len=T):
    """Scaled RoPE tables in chunk-major 2-head-packed bf16 layout."""
    ch = t_len // 128
    n = np.arange(N, dtype=np.float64)
    tq = np.floor(n / 2.0) * 2.0
    freqs = 1.0 / (THETA ** (tq / N)) / (2.0 * math.pi)
    t = np.arange(t_len, dtype=np.float64)[:, None]
    ang = ((t * freqs[None, :]) % 1.0) * (2.0 * math.pi)
    sc = float(N) ** -0.25
    cc = (np.cos(ang) * sc).astype(np.float32)
    ss = (np.sin(ang) * sc).astype(np.float32)
    ss[:, 0::2] *= -1.0

    def pack(x):  # [t, N] -> [128, ch*2*N]
        xc = x.reshape(ch, 128, N).transpose(1, 0, 2)       # [128, ch, N]
        x2 = np.repeat(xc[:, :, None, :], 2, axis=2)        # [128, ch, 2, N]
        return np.ascontiguousarray(x2.reshape(128, ch * 2 * N).astype(BF16))

    return pack(cc), pack(ss)


def _mask():
    # [key-part, query-free]: keep scores where key < query (strict causal)
    m = np.triu(np.ones((128, 128), dtype=np.float32), k=1)
    return np.ascontiguousarray(np.concatenate([m, m], axis=1).astype(BF16))


def _identity():
    return np.eye(128, dtype=BF16)


def _pack_pair(x, t_len=T):  # x [2, t, N] -> [128, ch*2*N] bf16, [p, (c,j,n)]
    ch = t_len // 128
    xc = x.reshape(2, ch, 128, N).transpose(2, 1, 0, 3)     # [128, ch, 2, N]
    return np.ascontiguousarray(xc.reshape(128, ch * 2 * N).astype(BF16))


def _stages(ch=CH):
    if ch <= 8:
        return [min(2, ch)] + ([ch - 2] if ch > 2 else [])
    out = [2, 3, 3]
    left = ch - 8
    while left > 0:
        out.append(min(8, left))
        left -= 8
    return out


def build_program(t_len=T, debug_stop=None):
    import concourse.mybir as mybir
    import concourse.tile as tile
    from concourse import bacc

    f32 = mybir.dt.float32
    bf = mybir.dt.bfloat16
    ch = t_len // 128
    W = ch * 128  # free width of a packed pair tensor

    nc = bacc.Bacc(None, target_bir_lowering=False)
    q = nc.dram_tensor("q", [NPAIR, 128, W], bf, kind="ExternalInput")
    qs = nc.dram_tensor("qs", [NPAIR, 128, W], bf, kind="ExternalInput")
    v = nc.dram_tensor("v", [NPAIR, 128, W], bf, kind="ExternalInput")
    cc = nc.dram_tensor("cc", [128, W], bf, kind="ExternalInput")
    ss = nc.dram_tensor("ss", [128, W], bf, kind="ExternalInput")
    mu = nc.dram_tensor("mu", [128, 256], bf, kind="ExternalInput")
    ident = nc.dram_tensor("ident", [128, 128], bf, kind="ExternalInput")
    o = nc.dram_tensor("o", [NPAIR, 128, W], bf, kind="ExternalOutput")

    with tile.TileContext(nc) as tc:
        with (
            tc.tile_pool(name="const", bufs=1) as constp,
            tc.tile_pool(name="pair", bufs=1) as pairp,
            tc.tile_pool(name="rope", bufs=3) as ropep,
            tc.tile_pool(name="work", bufs=3) as workp,
            tc.tile_pool(name="ost", bufs=2) as ostp,
            tc.tile_pool(name="ps", bufs=2, space="PSUM") as psp,
            tc.tile_pool(name="pso", bufs=2 if XBAR else 1,
                         space="PSUM") as psop,
            tc.tile_pool(name="psm", bufs=1, space="PSUM") as psmp,
        ):
            cc_sb = constp.tile([128, W], bf)
            ss_sb = constp.tile([128, W], bf)
            mu_sb = constp.tile([128, 256], bf)
            nc.sync.dma_start(mu_sb[:], mu[:])
            if not XBAR:
                id_sb = constp.tile([128, 128], bf)
                nc.sync.dma_start(id_sb[:], ident[:])
            nc.sync.dma_start(cc_sb[:], cc[:])
            nc.sync.dma_start(ss_sb[:], ss[:])

            qr2 = {}
            qr2x = {}
            qrt2 = {}
            qrtx2 = {}
            v_sb = {}
            for g in range(NPAIR):
                qr2[g] = pairp.tile([128, W], bf, name=f"qr{g}", tag=f"qr{g}")
                qr2x[g] = pairp.tile([128, W], bf, name=f"qrx{g}",
                                     tag=f"qrx{g}")
                qrt2[g] = pairp.tile([128, W], bf, name=f"qrt{g}",
                                     tag=f"qrt{g}")
                qrtx2[g] = pairp.tile([128, W], bf, name=f"qrtx{g}",
                                      tag=f"qrtx{g}")
                v_sb[g] = pairp.tile([128, W], bf, name=f"v{g}", tag=f"v{g}")

            # ---- RoPE + transpose, stage-pipelined, both pairs ----
            cbase = 0
            for stage in _stages(ch):
                fsl = slice(cbase * 128, (cbase + stage) * 128)
                for g in range(NPAIR):
                    fw = stage * 128
                    q_st = ropep.tile([128, 8 * 128], bf, name="qst",
                                      tag="q")[:, :fw]
                    qs_st = ropep.tile([128, 8 * 128], bf, name="qsst",
                                       tag="qs")[:, :fw]
                    nc.sync.dma_start(q_st, q[g][:, fsl])
                    nc.sync.dma_start(qs_st, qs[g][:, fsl])
                    nc.sync.dma_start(v_sb[g][:, fsl], v[g][:, fsl])

                    t1 = ropep.tile([128, 8 * 128], bf, name="t1",
                                    tag="t1")[:, :fw]
                    t2 = ropep.tile([128, 8 * 128], bf, name="t2",
                                    tag="t2")[:, :fw]
                    nc.vector.tensor_mul(t1, q_st, cc_sb[:, fsl])
                    nc.gpsimd.tensor_mul(t2, qs_st, ss_sb[:, fsl])
                    nc.vector.tensor_add(qr2[g][:, fsl], t1, t2)

                    if XBAR:
                        # j-swapped copy so head B's transposed strips land
                        # on partitions 0:64 (base-64 matmul operands hit a
                        # HW bug; everything must read from base 0)
                        qrv = qr2[g][:, fsl].rearrange(
                            "p (c j n) -> p c j n", j=2, n=N)
                        qxv = qr2x[g][:, fsl].rearrange(
                            "p (c j n) -> p c j n", j=2, n=N)
                        nc.vector.tensor_copy(qxv[:, :, 0], qrv[:, :, 1])
                        nc.vector.tensor_copy(qxv[:, :, 1], qrv[:, :, 0])

                    for c in range(cbase, cbase + stage):
                        csl = slice(c * 128, (c + 1) * 128)
                        if XBAR:
                            nc.sync.dma_start_transpose(
                                qrt2[g][:, csl], qr2[g][:, csl])
                            nc.scalar.dma_start_transpose(
                                qrtx2[g][:, csl], qr2x[g][:, csl])
                        else:
                            for j in range(2):
                                hsl = slice(c * 128 + j * 64,
                                            c * 128 + (j + 1) * 64)
                                tr_ps = psp.tile([64, 128], bf, name="tr",
                                                 tag="tr")
                                nc.tensor.transpose(
                                    tr_ps[:], qr2[g][:, hsl], id_sb[:])
                                dst = qrt2[g] if j == 0 else qrtx2[g]
                                nc.scalar.copy(dst[0:64, csl], tr_ps[:])
                cbase += stage

            if debug_stop == "rope":
                for g in range(NPAIR):
                    nc.sync.dma_start(o[g][:, :], qr2[g][:, :])
            # ---- main loop: both pairs interleaved chunk by chunk ----
            m_ps = {}
            mb_prev = {}
            ob = {}
            ost = {}
            for g in range(NPAIR if debug_stop != "rope" else 0):
                m_ps[g] = psmp.tile([64, 128], f32, name=f"m{g}",
                                    tag=f"m{g}", bufs=1)

            for c in range(ch if debug_stop != "rope" else 0):
                for g in range(NPAIR):
                    csl = slice(c * 128, (c + 1) * 128)
                    # intra scores: both heads, disjoint PE row groups
                    p2 = psp.tile([128, 256], f32, name="p2", tag="p")
                    pm = workp.tile([128, 256], bf, name="pm", tag="pm")
                    pmm = workp.tile([128, 256], bf, name="pmm", tag="pmm")
                    for j in range(2):
                        src_t = qrt2[g] if j == 0 else qrtx2[g]
                        strip = src_t[0:64, csl]
                        nc.tensor.matmul(
                            p2[:, j * 128:(j + 1) * 128], strip, strip,
                            start=True, stop=True,
                        )
                    nc.scalar.copy(pm[:], p2[:])
                    if debug_stop in ("nomask", "intra"):
                        pmm = pm
                    else:
                        nc.gpsimd.tensor_mul(pmm[:], pm[:], mu_sb[:])
                    if debug_stop in ("intra", "mask"):
                        if c % 8 == 0:
                            nc.sync.dma_start(
                                o[g][:, c * 128:(c + 2) * 128], pmm[:])
                        continue

                    # output accumulator: 4 chunks per PSUM bank
                    k4 = c % 4
                    if k4 == 0:
                        ob[g] = psop.tile([128, 512], f32, name=f"ob{g}", tag=f"ob{g}")
                        ost[g] = ostp.tile([128, 512], bf, name=f"ostt{g}", tag=f"ost{g}")
                    osl = ob[g][:, k4 * 128:(k4 + 1) * 128]

                    for j in range(2):
                        jsl = slice(j * 64, (j + 1) * 64)
                        hsl = slice(c * 128 + j * 64, c * 128 + (j + 1) * 64)
                        src_t = qrt2[g] if j == 0 else qrtx2[g]
                        strip = src_t[0:64, csl]
                        if c > 0 and debug_stop not in ("nointer", "nostate"):
                            nc.tensor.matmul(
                                osl[:, jsl], strip, mb_prev[g][:, jsl],
                                start=True, stop=False,
                            )
                        nc.tensor.matmul(
                            osl[:, jsl], pmm[:, j * 128:(j + 1) * 128],
                            v_sb[g][:, hsl],
                            start=(c == 0 or debug_stop in
                                   ("nointer", "nostate")), stop=True,
                        )
                        # state: M += QR_c^T @ V_c (col-tiled: head j ->
                        # output partitions j*64..)
                        if debug_stop != "nostate":
                            nc.tensor.matmul(
                                m_ps[g][:, jsl],
                                qr2[g][:, hsl], v_sb[g][:, hsl],
                                start=(c == 0 and j == 0),
                                stop=(c == ch - 1),
                                skip_group_check=True,
                            )
                    if c < ch - 1 and debug_stop != "nostate":
                        mb = workp.tile([64, 128], bf, name=f"mb{g}", tag=f"mb{g}")
                        if c % 2 == 0:
                            nc.scalar.copy(mb[:], m_ps[g][:])
                        else:
                            nc.vector.tensor_copy(mb[:], m_ps[g][:])
                        mb_prev[g] = mb

                    if k4 == 3:
                        nc.vector.tensor_copy(ost[g][:], ob[g][:])
                        c0 = (c // 4) * 4
                        nc.sync.dma_start(
                            o[g][:, c0 * 128:(c0 + 4) * 128], ost[g][:])

    nc.compile()
    return nc


_CACHE = {}


def _get_program():
    if "nc" not in _CACHE:
        _CACHE["nc"] = build_program()
    return _CACHE["nc"]


def _in_maps(Q, V):
    """Host marshaling: full fp32 inputs -> per-core bf16 input maps."""
    Q = np.asarray(Q, dtype=np.float32).reshape(NCORES, HPC, T, N)
    V = np.asarray(V, dtype=np.float32).reshape(NCORES, HPC, T, N)
    # swap feature pairs (sign lives in the ss table)
    Qsw = np.ascontiguousarray(
        Q.reshape(NCORES, HPC, T, N // 2, 2)[..., ::-1]
    ).reshape(NCORES, HPC, T, N)
    cc, ss = _host_tables()
    mu = _mask()
    ident = _identity()
    maps = []
    for i in range(NCORES):
        qp = np.stack([_pack_pair(Q[i, 2 * g:2 * g + 2]) for g in range(NPAIR)])
        qsp = np.stack(
            [_pack_pair(Qsw[i, 2 * g:2 * g + 2]) for g in range(NPAIR)])
        vp = np.stack([_pack_pair(V[i, 2 * g:2 * g + 2]) for g in range(NPAIR)])
        maps.append({"q": qp, "qs": qsp, "v": vp, "cc": cc, "ss": ss,
                     "mu": mu, "ident": ident})
    return maps


def _unpack_out(results):
    """Per-core bf16 'o' tensors [NPAIR,128,CH*128] -> [B,H,T,N] fp32."""
    outs = []
    for r in results:
        x = np.asarray(r["o"]).reshape(NPAIR, 128, CH, 2, N)
        x = x.transpose(0, 3, 2, 1, 4).reshape(HPC, T, N)
        outs.append(x)
    return np.stack(outs).reshape(B, H, T, N).astype(np.float32)


def kernel(Q, V):
    from concourse.bass_utils import run_bass_kernel_spmd

    nc = _get_program()
    in_maps = _in_maps(Q, V)
    res = run_bass_kernel_spmd(nc, in_maps, core_ids=list(range(NCORES)))
    return _unpack_out(res.results)


# revision 15
# speedup vs baseline: 1.3206x; 1.3206x over previous
"""Trainium2 Bass kernel for nn_Attention_23424751632639.

Computation (per (b,h)):  out = tril_strict(rope(Q) @ rope(Q).T / sqrt(N)) @ V
Chunked linear attention (exact reordering of the sums):
  out_c = QR_c @ M_c  +  strict_mask(QR_c @ QR_c^T) @ V_c
  M_{c+1} = M_c + QR_c^T @ V_c          (M is the [64,64] running state)
with QR = rope(Q) * N**-0.25 (scale folded into the cos/sin tables).

v2 design vs baseline:
  * Host marshals Q, swap(Q), V to bf16 in chunk-major [128, ch*2*64]
    layout with 2 heads packed per tensor -> contiguous 4-8KB DMA lines
    (the fp32 rearrange DMAs were 256B/partition packets at ~110GB/s).
  * RoPE on device in bf16: DVE 2x-mode muls/adds + GpSimd for the
    second multiply; swap(Q) arrives pre-permuted so no strided copies.
  * QR^T strips produced by DMA-xbar transpose (dma_start_transpose of
    [128,128] 2-head feature blocks), not PE transposes + ACT evac.
  * intra/inter matmuls of the two packed heads use disjoint PE row
    groups (K=64 at base partitions 0/64) -> concurrent tiles.
  * state matmuls col-tiled into one [128,64] PSUM tile (head A on
    partitions 0:64, head B on 64:128) -> one snapshot copy per chunk
    serves both heads' inter matmuls.
  * mask applied as ACT PSUM->SBUF bf16 copy + GpSimd bf16 multiply.
  * outputs accumulate 4 chunks per PSUM bank; single DVE bf16 evac and
    one 512-col DMA per 4 chunks; output returned to HBM in bf16
    (tolerance is 2e-2; bf16 out adds ~4e-3).

Sharding: B*H = 32 (b,h) pairs -> 4 per core across 8 cores; no collectives.
"""

import math
import sys

import numpy as np

if "/opt/trn_rl_repo" not in sys.path:
    sys.path.insert(0, "/opt/trn_rl_repo")

import ml_dtypes

BF16 = ml_dtypes.bfloat16

B, H, T, N = 2, 16, 4096, 64
THETA = 2.0 ** 16
NCORES = 8
HPC = (B * H) // NCORES      # heads per core (4)
NPAIR = HPC // 2             # head pairs per core (2)
CH = T // 128                # 128-row chunks per head (32)
XBAR = True                  # transpose via DMA xbar (False -> PE transpose)


def _host_tables(t_len=T):
    """Scaled RoPE tables in chunk-major 2-head-packed bf16 layout."""
    ch = t_len // 128
    n = np.arange(N, dtype=np.float64)
    tq = np.floor(n / 2.0) * 2.0
    freqs = 1.0 / (THETA ** (tq / N)) / (2.0 * math.pi)
    t = np.arange(t_len, dtype=np.float64)[:, None]
    ang = ((t * freqs[None, :]) % 1.0) * (2.0 * math.pi)
    sc = float(N) ** -0.25
    cc = (np.cos(ang) * sc).astype(np.float32)
    ss = (np.sin(ang) * sc).astype(np.float32)
    ss[:, 0::2] *= -1.0

    def pack(x):  # [t, N] -> [128, ch*2*N]
        xc = x.reshape(ch, 128, N).transpose(1, 0, 2)       # [128, ch, N]
        x2 = np.repeat(xc[:, :, None, :], 2, axis=2)        # [128, ch, 2, N]
        return np.ascontiguousarray(x2.reshape(128, ch * 2 * N).astype(BF16))

    return pack(cc), pack(ss)


def _mask():
    # [key-part, query-free]: keep scores where key < query (strict causal)
    m = np.triu(np.ones((128, 128), dtype=np.float32), k=1)
    return np.ascontiguousarray(np.concatenate([m, m], axis=1).astype(BF16))


def _identity():
    return np.eye(128, dtype=BF16)


def _pack_pair(x, t_len=T):  # x [2, t, N] -> [128, ch*2*N] bf16, [p, (c,j,n)]
    ch = t_len // 128
    xc = x.reshape(2, ch, 128, N).transpose(2, 1, 0, 3)     # [128, ch, 2, N]
    return np.ascontiguousarray(xc.reshape(128, ch * 2 * N).astype(BF16))


def _stages(ch=CH):
    if ch <= 8:
        return [min(2, ch)] + ([ch - 2] if ch > 2 else [])
    out = [2, 3, 3]
    left = ch - 8
    while left > 0:
        out.append(min(8, left))
        left -= 8
    return out


def build_program(t_len=T, debug_stop=None):
    import concourse.mybir as mybir
    import concourse.tile as tile
    from concourse import bacc

    f32 = mybir.dt.float32
    bf = mybir.dt.bfloat16
    ch = t_len // 128
    W = ch * 128  # free width of a packed pair tensor

    nc = bacc.Bacc(None, target_bir_lowering=False)
    q = nc.dram_tensor("q", [NPAIR, 128, W], bf, kind="ExternalInput")
    qs = nc.dram_tensor("qs", [NPAIR, 128, W], bf, kind="ExternalInput")
    v = nc.dram_tensor("v", [NPAIR, 128, W], bf, kind="ExternalInput")
    cc = nc.dram_tensor("cc", [128, W], bf, kind="ExternalInput")
    ss = nc.dram_tensor("ss", [128, W], bf, kind="ExternalInput")
    mu = nc.dram_tensor("mu", [128, 256], bf, kind="ExternalInput")
    ident = nc.dram_tensor("ident", [128, 128], bf, kind="ExternalInput")
    o = nc.dram_tensor("o", [NPAIR, 128, W], bf, kind="ExternalOutput")

    with tile.TileContext(nc) as tc:
        with (
            tc.tile_pool(name="const", bufs=1) as constp,
            tc.tile_pool(name="pair", bufs=1) as pairp,
            tc.tile_pool(name="rope", bufs=3) as ropep,
            tc.tile_pool(name="work", bufs=3) as workp,
            tc.tile_pool(name="ost", bufs=2) as ostp,
            tc.tile_pool(name="ps", bufs=2, space="PSUM") as psp,
            tc.tile_pool(name="pso", bufs=2 if XBAR else 1,
                         space="PSUM") as psop,
            tc.tile_pool(name="psm", bufs=1, space="PSUM") as psmp,
        ):
            cc_sb = constp.tile([128, W], bf)
            ss_sb = constp.tile([128, W], bf)
            mu_sb = constp.tile([128, 256], bf)
            nc.sync.dma_start(mu_sb[:], mu[:])
            if not XBAR:
                id_sb = constp.tile([128, 128], bf)
                nc.sync.dma_start(id_sb[:], ident[:])
            nc.sync.dma_start(cc_sb[:], cc[:])
            nc.sync.dma_start(ss_sb[:], ss[:])

            qr2 = {}
            qr2x = {}
            qrt2 = {}
            qrtx2 = {}
            v_sb = {}
            for g in range(NPAIR):
                qr2[g] = pairp.tile([128, W], bf, name=f"qr{g}", tag=f"qr{g}")
                qr2x[g] = pairp.tile([128, W], bf, name=f"qrx{g}",
                                     tag=f"qrx{g}")
                qrt2[g] = pairp.tile([128, W], bf, name=f"qrt{g}",
                                     tag=f"qrt{g}")
                qrtx2[g] = pairp.tile([128, W], bf, name=f"qrtx{g}",
                                      tag=f"qrtx{g}")
                v_sb[g] = pairp.tile([128, W], bf, name=f"v{g}", tag=f"v{g}")

            # ---- RoPE, pair-major so pair 0 finishes early; input DMAs
            # ride the sync HWDGE ring, batched transposes the scalar ring.
            for g in range(NPAIR):
                cbase = 0
                for stage in _stages(ch):
                    fsl = slice(cbase * 128, (cbase + stage) * 128)
                    fw = stage * 128
                    q_st = ropep.tile([128, 8 * 128], bf, name="qst",
                                      tag="q")[:, :fw]
                    qs_st = ropep.tile([128, 8 * 128], bf, name="qsst",
                                       tag="qs")[:, :fw]
                    nc.sync.dma_start(q_st, q[g][:, fsl])
                    nc.sync.dma_start(qs_st, qs[g][:, fsl])
                    nc.sync.dma_start(v_sb[g][:, fsl], v[g][:, fsl])

                    t1 = ropep.tile([128, 8 * 128], bf, name="t1",
                                    tag="t1")[:, :fw]
                    t2 = ropep.tile([128, 8 * 128], bf, name="t2",
                                    tag="t2")[:, :fw]
                    nc.gpsimd.tensor_mul(t1, q_st, cc_sb[:, fsl])
                    nc.gpsimd.tensor_mul(t2, qs_st, ss_sb[:, fsl])
                    nc.vector.tensor_add(qr2[g][:, fsl], t1, t2)

                    if XBAR:
                        # j-swapped copy so head B's transposed strips land
                        # on partitions 0:64 (base-64 matmul operands hit a
                        # HW bug; everything must read from base 0)
                        qrv = qr2[g][:, fsl].rearrange(
                            "p (c j n) -> p c j n", j=2, n=N)
                        qxv = qr2x[g][:, fsl].rearrange(
                            "p (c j n) -> p c j n", j=2, n=N)
                        nc.vector.tensor_copy(qxv[:, :, 0], qrv[:, :, 1])
                        nc.vector.tensor_copy(qxv[:, :, 1], qrv[:, :, 0])
                    else:
                        for c in range(cbase, cbase + stage):
                            csl = slice(c * 128, (c + 1) * 128)
                            for j in range(2):
                                hsl = slice(c * 128 + j * 64,
                                            c * 128 + (j + 1) * 64)
                                tr_ps = psp.tile([64, 128], bf, name="tr",
                                                 tag="tr")
                                nc.tensor.transpose(
                                    tr_ps[:], qr2[g][:, hsl], id_sb[:])
                                dst = qrt2[g] if j == 0 else qrtx2[g]
                                nc.scalar.copy(dst[0:64, csl], tr_ps[:])
                    cbase += stage
                if XBAR:
                    # one blocked whole-tile transpose per source tile:
                    # out[p, c, t] = src[t, c*128 + p]
                    nc.scalar.dma_start_transpose(
                        qrt2[g].rearrange("p (c t) -> p c t", c=ch),
                        qr2[g][:, :])
                    nc.scalar.dma_start_transpose(
                        qrtx2[g].rearrange("p (c t) -> p c t", c=ch),
                        qr2x[g][:, :])

            if debug_stop == "rope":
                for g in range(NPAIR):
                    nc.sync.dma_start(o[g][:, :], qr2[g][:, :])
            # ---- main loop: both pairs interleaved chunk by chunk ----
            m_ps = {}
            mb_prev = {}
            ob = {}
            ost = {}
            for g in range(NPAIR if debug_stop != "rope" else 0):
                m_ps[g] = psmp.tile([64, 128], f32, name=f"m{g}",
                                    tag=f"m{g}", bufs=1)

            for c in range(ch if debug_stop != "rope" else 0):
                for g in range(NPAIR):
                    csl = slice(c * 128, (c + 1) * 128)
                    # intra scores: both heads, disjoint PE row groups
                    p2 = psp.tile([128, 256], f32, name="p2", tag="p")
                    pmm = workp.tile([128, 256], bf, name="pmm", tag="pmm")
                    for j in range(2):
                        src_t = qrt2[g] if j == 0 else qrtx2[g]
                        strip = src_t[0:64, csl]
                        nc.tensor.matmul(
                            p2[:, j * 128:(j + 1) * 128], strip, strip,
                            start=True, stop=True,
                        )
                    if c % 2 == 0:
                        nc.vector.scalar_tensor_tensor(
                            pmm[:], p2[:], 1.0, mu_sb[:],
                            mybir.AluOpType.mult, mybir.AluOpType.mult)
                    else:
                        pm = workp.tile([128, 256], bf, name="pm", tag="pm")
                        nc.scalar.copy(pm[:], p2[:])
                        nc.vector.tensor_mul(pmm[:], pm[:], mu_sb[:])

                    # output accumulator: 4 chunks per PSUM bank
                    k4 = c % 4
                    if k4 == 0:
                        ob[g] = psop.tile([128, 512], f32, name=f"ob{g}", tag=f"ob{g}")
                        ost[g] = ostp.tile([128, 512], bf, name=f"ostt{g}", tag=f"ost{g}")
                    osl = ob[g][:, k4 * 128:(k4 + 1) * 128]

                    for j in range(2):
                        jsl = slice(j * 64, (j + 1) * 64)
                        hsl = slice(c * 128 + j * 64, c * 128 + (j + 1) * 64)
                        src_t = qrt2[g] if j == 0 else qrtx2[g]
                        strip = src_t[0:64, csl]
                        if c > 0 and debug_stop not in ("nointer", "nostate"):
                            nc.tensor.matmul(
                                osl[:, jsl], strip, mb_prev[g][:, jsl],
                                start=True, stop=False,
                            )
                        nc.tensor.matmul(
                            osl[:, jsl], pmm[:, j * 128:(j + 1) * 128],
                            v_sb[g][:, hsl],
                            start=(c == 0 or debug_stop in
                                   ("nointer", "nostate")), stop=True,
                        )
                        # state: M += QR_c^T @ V_c (col-tiled: head j ->
                        # output partitions j*64..)
                        if debug_stop != "nostate":
                            nc.tensor.matmul(
                                m_ps[g][:, jsl],
                                qr2[g][:, hsl], v_sb[g][:, hsl],
                                start=(c == 0 and j == 0),
                                stop=(c == ch - 1),
                                skip_group_check=True,
                            )
                    if c < ch - 1 and debug_stop != "nostate":
                        mb = workp.tile([64, 128], bf, name=f"mb{g}", tag=f"mb{g}")
                        nc.scalar.copy(mb[:], m_ps[g][:])
                        mb_prev[g] = mb

                    if k4 == 3:
                        nc.scalar.copy(ost[g][:], ob[g][:])
                        c0 = (c // 4) * 4
                        nc.sync.dma_start(
                            o[g][:, c0 * 128:(c0 + 4) * 128], ost[g][:])

    nc.compile()
    return nc


_CACHE = {}


def _get_program():
    if "nc" not in _CACHE:
        _CACHE["nc"] = build_program()
    return _CACHE["nc"]


def _in_maps(Q, V):
    """Host marshaling: full fp32 inputs -> per-core bf16 input maps."""
    Q = np.asarray(Q, dtype=np.float32).reshape(NCORES, HPC, T, N)
    V = np.asarray(V, dtype=np.float32).reshape(NCORES, HPC, T, N)
    # swap feature pairs (sign lives in the ss table)
    Qsw = np.ascontiguousarray(
        Q.reshape(NCORES, HPC, T, N // 2, 2)[..., ::-1]
    ).reshape(NCORES, HPC, T, N)
    cc, ss = _host_tables()
    mu = _mask()
    ident = _identity()
    maps = []
    for i in range(NCORES):
        qp = np.stack([_pack_pair(Q[i, 2 * g:2 * g + 2]) for g in range(NPAIR)])
        qsp = np.stack(
            [_pack_pair(Qsw[i, 2 * g:2 * g + 2]) for g in range(NPAIR)])
        vp = np.stack([_pack_pair(V[i, 2 * g:2 * g + 2]) for g in range(NPAIR)])
        maps.append({"q": qp, "qs": qsp, "v": vp, "cc": cc, "ss": ss,
                     "mu": mu, "ident": ident})
    return maps


def _unpack_out(results):
    """Per-core bf16 'o' tensors [NPAIR,128,CH*128] -> [B,H,T,N] fp32."""
    outs = []
    for r in results:
        x = np.asarray(r["o"]).reshape(NPAIR, 128, CH, 2, N)
        x = x.transpose(0, 3, 2, 1, 4).reshape(HPC, T, N)
        outs.append(x)
    return np.stack(outs).reshape(B, H, T, N).astype(np.float32)


def kernel(Q, V):
    from concourse.bass_utils import run_bass_kernel_spmd

    nc = _get_program()
    in_maps = _in_maps(Q, V)
    res = run_bass_kernel_spmd(nc, in_maps, core_ids=list(range(NCORES)))
    return _unpack_out(res.results)
